# revision 4
# baseline (speedup 1.0000x reference)
"""Trainium2 Bass kernel for nn_Decoder: teacher-forced RNN decoder.

B=512, L=111, E=256, H=512, V=512. Data-parallel over batch: 8 cores x 64 rows.

Per-core layout (all matmul operands transposed so the contraction dim is on
partitions):
  - h kept as (H x B) tiles (4 x [128, 64], bf16), full history in SBUF
  - recurrence: psum[m] = sum_k W_hhT[k, m-block].T @ h[k]  (16 MMs/step)
  - input projection: xs = W_e2h[token] via one-hot matmul, batched over
    8-step chunks (W_e2h = W_embd @ W_ih.T computed on device in fp32)
  - psum += xs (DVE), h_new = tanh(psum + bias) (ACT, per-partition bias)
  - output projection per 2 steps: logits = h2.T @ W_outT + b_out with
    lhsT = two h columns blocks (M=128), N=V=512
"""

import sys
import os

sys.path.insert(0, "/opt/trn_rl_repo")

from contextlib import ExitStack

import numpy as np
import ml_dtypes

import concourse.bass as bass
import concourse.tile as tile
import concourse.mybir as mybir
from concourse import bacc
from concourse.bass_utils import run_bass_kernel_spmd

# ---------------------------------------------------------------------------

N_CORES = 8
B_FULL = 512
B = B_FULL // N_CORES  # 64 rows per core
L = 111
V = 512
E = 256
H = 512
P = 128
KH = H // P  # 4 h-tiles
KV = V // P  # 4 v-tiles
KE = E // P  # 2 e-tiles
CH = 8  # steps per input-projection chunk

F32 = mybir.dt.float32
BF16 = mybir.dt.bfloat16
I32 = mybir.dt.int32

_CACHE = {}


def _build_bass():
    nc = bacc.Bacc("TRN2", target_bir_lowering=False, debug=False)

    d_tok = nc.dram_tensor("tok", [P, L * B], F32, kind="ExternalInput").ap()
    d_ctxT = nc.dram_tensor("ctxT", [P, KH, B], BF16, kind="ExternalInput").ap()
    d_whhT = nc.dram_tensor("whhT", [P, KH, H], BF16, kind="ExternalInput").ap()
    d_woutT = nc.dram_tensor("woutT", [P, KH, V], BF16, kind="ExternalInput").ap()
    d_wembdT = nc.dram_tensor("wembdT", [P, KE, V], F32, kind="ExternalInput").ap()
    d_wihT = nc.dram_tensor("wihT", [P, KE, H], F32, kind="ExternalInput").ap()
    d_bias = nc.dram_tensor("bias", [P, KH], F32, kind="ExternalInput").ap()
    d_bout = nc.dram_tensor("bout", [P, V], BF16, kind="ExternalInput").ap()
    d_out = nc.dram_tensor("out", [B, L * V], F32, kind="ExternalOutput").ap()
    out3 = d_out.rearrange("b (l v) -> b l v", v=V)

    with tile.TileContext(nc) as tc:
        with ExitStack() as ctx:
            consts = ctx.enter_context(tc.tile_pool(name="consts", bufs=1))
            hpool = ctx.enter_context(tc.tile_pool(name="hist", bufs=1))
            tokp = ctx.enter_context(tc.tile_pool(name="tok", bufs=2))
            ohp = ctx.enter_context(tc.tile_pool(name="oh", bufs=2))
            xsp = ctx.enter_context(tc.tile_pool(name="xs", bufs=2))
            stgp = ctx.enter_context(tc.tile_pool(name="stg", bufs=4))
            ps_h = ctx.enter_context(tc.tile_pool(name="psh", bufs=1, space="PSUM"))
            ps_xs = ctx.enter_context(tc.tile_pool(name="psxs", bufs=2, space="PSUM"))
            ps_o = ctx.enter_context(tc.tile_pool(name="pso", bufs=2, space="PSUM"))

            # ---- constants to SBUF ----
            whhT = consts.tile([P, KH, H], BF16)
            nc.sync.dma_start(whhT[:], d_whhT)
            woutT = consts.tile([P, KH, V], BF16)
            nc.sync.dma_start(woutT[:], d_woutT)
            wembdT = consts.tile([P, KE, V], F32)
            nc.sync.dma_start(wembdT[:], d_wembdT)
            wihT = consts.tile([P, KE, H], F32)
            nc.sync.dma_start(wihT[:], d_wihT)
            bias_sb = consts.tile([P, KH], F32)
            nc.sync.dma_start(bias_sb[:], d_bias)
            bout_sb = consts.tile([P, V], BF16)
            nc.sync.dma_start(bout_sb[:], d_bout)
            iota_sb = consts.tile([P, KV], F32)
            nc.gpsimd.iota(
                iota_sb[:],
                pattern=[[P, KV]],
                base=0,
                channel_multiplier=1,
                allow_small_or_imprecise_dtypes=True,
            )

            # ---- W_e2h = W_embd @ W_ih.T, kept bf16 as one-hot lhsT ----
            # we2h[p, kv, h] = W_e2h[kv*128 + p, h]
            we2h = consts.tile([P, KV, H], BF16)
            for kv in range(KV):
                pw = ps_xs.tile([P, H], F32, tag="xs")
                for ke in range(KE):
                    nc.tensor.matmul(
                        pw[:],
                        wembdT[:, ke, kv * P : (kv + 1) * P],
                        wihT[:, ke, :],
                        start=(ke == 0),
                        stop=(ke == KE - 1),
                    )
                nc.vector.tensor_copy(out=we2h[:, kv, :], in_=pw[:])

            # ---- hidden state history: slot 0 = context, slot t+1 = h_t ----
            h_hist = hpool.tile([P, KH, (L + 1) * B], BF16)
            nc.sync.dma_start(h_hist[:, :, 0:B], d_ctxT)

            # recurrence psum banks, one per h-tile
            psum_h = [
                ps_h.tile([P, B], F32, tag=f"ph{m}", name=f"psum_h{m}")
                for m in range(KH)
            ]

            # chunk boundaries
            chunk_starts = list(range(0, L, CH))

            for t0 in chunk_starts:
                n_steps = min(CH, L - t0)
                n = n_steps * B

                # tokens for the chunk (replicated across partitions on host)
                tok_t = tokp.tile([P, CH * B], F32, tag="tok")
                nc.sync.dma_start(tok_t[:, :n], d_tok[:, t0 * B : t0 * B + n])

                # one-hot: oh[p, kv, c] = (tok[c] == kv*128 + p)
                oh = ohp.tile([P, KV, CH * B], BF16, tag="oh")
                for kv in range(KV):
                    nc.vector.tensor_scalar(
                        oh[:, kv, :n],
                        tok_t[:, :n],
                        iota_sb[:, kv : kv + 1],
                        None,
                        mybir.AluOpType.is_equal,
                    )

                # xs[m] = sum_kv we2h[kv, m-block].T @ oh[kv]  -> (128, n)
                xs = xsp.tile([P, KH, CH * B], BF16, tag="xs")
                for m in range(KH):
                    pxs = ps_xs.tile([P, CH * B], F32, tag="xs")
                    for kv in range(KV):
                        nc.tensor.matmul(
                            pxs[:, :n],
                            we2h[:, kv, m * P : (m + 1) * P],
                            oh[:, kv, :n],
                            start=(kv == 0),
                            stop=(kv == KV - 1),
                        )
                    nc.vector.tensor_copy(out=xs[:, m, :n], in_=pxs[:, :n])

                # ---- recurrence steps of this chunk ----
                for t in range(t0, t0 + n_steps):
                    c0 = (t - t0) * B
                    for m in range(KH):
                        for k in range(KH):
                            nc.tensor.matmul(
                                psum_h[m][:],
                                whhT[:, k, m * P : (m + 1) * P],
                                h_hist[:, k, t * B : (t + 1) * B],
                                start=(k == 0),
                                stop=(k == KH - 1),
                            )
                        nc.vector.tensor_tensor(
                            psum_h[m][:],
                            psum_h[m][:],
                            xs[:, m, c0 : c0 + B],
                            mybir.AluOpType.add,
                        )
                        nc.scalar.activation(
                            h_hist[:, m, (t + 1) * B : (t + 2) * B],
                            psum_h[m][:],
                            mybir.ActivationFunctionType.Tanh,
                            bias=bias_sb[:, m : m + 1],
                        )

                    # ---- output projection for the completed pair ----
                    if t % 2 == 1:
                        ta = t - 1  # pair covers steps (ta, ta+1)
                        po = ps_o.tile([P, V], F32, tag="op")
                        for k in range(KH):
                            nc.tensor.matmul(
                                po[:],
                                h_hist[:, k, (ta + 1) * B : (ta + 3) * B],
                                woutT[:, k, :],
                                start=(k == 0),
                                stop=(k == KH - 1),
                            )
                        stg = stgp.tile([P, V], F32, tag="stg")
                        nc.vector.tensor_tensor(
                            stg[:], po[:], bout_sb[:], mybir.AluOpType.add
                        )
                        nc.sync.dma_start(out3[:, ta, :], stg[0:B, :])
                        nc.sync.dma_start(out3[:, ta + 1, :], stg[B : 2 * B, :])

            # ---- last (odd) step 110: single-step output projection ----
            t = L - 1
            po = ps_o.tile([P, V], F32, tag="op")
            for k in range(KH):
                nc.tensor.matmul(
                    po[0:B, :],
                    h_hist[:, k, (t + 1) * B : (t + 2) * B],
                    woutT[:, k, :],
                    start=(k == 0),
                    stop=(k == KH - 1),
                )
            stg = stgp.tile([P, V], F32, tag="stg")
            nc.vector.tensor_tensor(
                stg[0:B, :], po[0:B, :], bout_sb[0:B, :], mybir.AluOpType.add
            )
            nc.sync.dma_start(out3[:, t, :], stg[0:B, :])

    nc.compile()
    return nc


def _bf(x):
    return np.ascontiguousarray(x.astype(ml_dtypes.bfloat16))


def _prep_inputs(x, context, target_teacher, W_embd, W_ih, W_hh, b_ih, b_hh,
                 W_out, b_out):
    """Host-side sharding / layout prep. Returns per-core input maps."""
    tt = np.asarray(target_teacher)
    tok_full = np.concatenate(
        [np.ones((B_FULL, 1), np.int32), tt[:, : L - 1].astype(np.int32)], axis=1
    )  # (B_FULL, L)

    W_hh = np.asarray(W_hh, np.float32)
    W_out = np.asarray(W_out, np.float32)
    W_embd = np.asarray(W_embd, np.float32)
    W_ih = np.asarray(W_ih, np.float32)
    context = np.asarray(context, np.float32)

    whhT = _bf(W_hh.T.reshape(KH, P, H).transpose(1, 0, 2))
    woutT = _bf(W_out.T.reshape(KH, P, V).transpose(1, 0, 2))
    wembdT = np.ascontiguousarray(
        W_embd.T.reshape(KE, P, V).transpose(1, 0, 2), np.float32
    )
    wihT = np.ascontiguousarray(
        W_ih.T.reshape(KE, P, H).transpose(1, 0, 2), np.float32
    )
    bias = np.ascontiguousarray(
        (np.asarray(b_ih, np.float32) + np.asarray(b_hh, np.float32))
        .reshape(KH, P)
        .T,
        np.float32,
    )
    bout = np.ascontiguousarray(
        np.broadcast_to(np.asarray(b_out, np.float32), (P, V))
    )
    bout = _bf(bout)

    in_maps = []
    for c in range(N_CORES):
        b0 = c * B
        tok_c = tok_full[b0 : b0 + B]  # (B, L)
        cols = np.ascontiguousarray(tok_c.T.reshape(-1), np.float32)  # (L*B,)
        tok_rep = np.ascontiguousarray(np.broadcast_to(cols, (P, L * B)))
        ctxT = _bf(
            context[b0 : b0 + B].T.reshape(KH, P, B).transpose(1, 0, 2)
        )
        in_maps.append(
            {
                "tok": tok_rep,
                "ctxT": ctxT,
                "whhT": whhT,
                "woutT": woutT,
                "wembdT": wembdT,
                "wihT": wihT,
                "bias": bias,
                "bout": bout,
            }
        )
    return in_maps


def kernel(**inputs):
    x = np.asarray(inputs["x"])
    assert x.shape[0] == B_FULL
    ml = int(np.asarray(inputs["max_length"]))
    assert ml == L, f"kernel hardcoded for max_length={L}, got {ml}"

    if "nc" not in _CACHE:
        _CACHE["nc"] = _build_bass()
    nc = _CACHE["nc"]

    in_maps = _prep_inputs(
        x,
        inputs["context"],
        inputs["target_teacher"],
        inputs["W_embd"],
        inputs["W_ih"],
        inputs["W_hh"],
        inputs["b_ih"],
        inputs["b_hh"],
        inputs["W_out"],
        inputs["b_out"],
    )
    res = run_bass_kernel_spmd(nc, in_maps, list(range(N_CORES)))
    out = np.empty((B_FULL, L * V), np.float32)
    for c in range(N_CORES):
        out[c * B : (c + 1) * B] = res.results[c]["out"]
    return out
